# revision 3
# baseline (speedup 1.0000x reference)
"""CharRNN Trainium2 kernel (data-parallel over batch, 8 NeuronCores).

Math (per core, batch slice of BC=16 rows):
  Mtable = embedding @ W_e + b_h                      [V, H]   (host)
  z_t^T  = W_h^T h_t^T + Mtable^T onehot(x_t)^T       [H, BC]
  h_{t+1}^T = tanh(z_t^T)
  logits_t^T = W_o^T h_{t+1}^T                        [V, BC]

Everything on device runs in the transposed domain so the hidden state
lives as [H(partitions), BC(free)] and tanh/matmuls use full partitions.
The recurrence is PE weight-load bound: per step 8x8 W_h blocks + 8
Mtable blocks are loaded as stationary operands with the BC-column state
as the moving operand.  W_o is applied every G steps over the SBUF-held
history ring with N=G*BC moving columns (weight reuse).
"""

import os
import sys

import numpy as np

_REPO = "/opt/trn_rl_repo"
if os.path.isdir(_REPO) and _REPO not in sys.path:
    sys.path.insert(0, _REPO)

import ml_dtypes  # noqa: E402

import concourse.bass as bass  # noqa: E402
import concourse.mybir as mybir  # noqa: E402
import concourse.tile as tile  # noqa: E402
from concourse import bacc  # noqa: E402
from concourse.bass import ds  # noqa: E402
from concourse.bass_utils import run_bass_kernel_spmd  # noqa: E402

B, L, V, E, H = 128, 1024, 32, 256, 1024
NCORES = 8
BC = B // NCORES  # 16 batch rows per core
KH = H // 128  # 8 contraction chunks
MH = H // 128  # 8 output chunks
G = 32  # steps per W_o flush; G*BC = 512 = max moving free dim
F32 = mybir.dt.float32

# dtype knobs: weights/state precision on device
DT_W = mybir.dt.bfloat16  # W_h / Mtable / W_o / onehot
DT_H = mybir.dt.bfloat16  # hidden-state history ring
UNROLL = "loop"  # "loop" (For_i over groups) | "full" (static unroll)

_cache = {}


def _np_dt(dt):
    return {F32: np.float32, mybir.dt.bfloat16: ml_dtypes.bfloat16}[dt]


def _build(L_=L, dt_w=DT_W, dt_h=DT_H, unroll=UNROLL):
    key = (L_, dt_w, dt_h, unroll)
    if key in _cache:
        return _cache[key]
    Gl = min(G, L_)
    NG = L_ // Gl
    assert NG * Gl == L_

    nc = bacc.Bacc(
        "TRN2", target_bir_lowering=False, debug=False, num_devices=NCORES
    )
    wh_d = nc.dram_tensor("wh", [128, KH * H], dt_w, kind="ExternalInput").ap()
    mt_d = nc.dram_tensor("mt", [V, H], dt_w, kind="ExternalInput").ap()
    wo_d = nc.dram_tensor("wo", [128, KH * V], dt_w, kind="ExternalInput").ap()
    oh_d = nc.dram_tensor("oh", [V, L_ * BC], dt_w, kind="ExternalInput").ap()
    h0_d = nc.dram_tensor("h0", [128, KH * BC], dt_h, kind="ExternalInput").ap()
    lg_d = nc.dram_tensor("logits", [V, L_ * BC], F32, kind="ExternalOutput").ap()
    hf_d = nc.dram_tensor("hfinal", [128, KH * BC], dt_h, kind="ExternalOutput").ap()

    with tile.TileContext(nc) as tc:
        with (
            tc.tile_pool(name="const", bufs=1) as cp,
            tc.tile_pool(name="stage", bufs=2) as sp,
            tc.tile_pool(name="psz", bufs=2, space="PSUM") as pz,
            tc.tile_pool(name="psl", bufs=2, space="PSUM") as pl,
        ):
            wh_s = cp.tile([128, KH * H], dt_w, tag="wh")
            mt_s = cp.tile([V, H], dt_w, tag="mt")
            wo_s = cp.tile([128, KH * V], dt_w, tag="wo")
            hist = cp.tile([128, Gl * 128], dt_h, tag="hist")
            nc.sync.dma_start(wh_s[:], wh_d)
            nc.sync.dma_start(mt_s[:], mt_d)
            nc.sync.dma_start(wo_s[:], wo_d)
            nc.sync.dma_start(hist[:, (Gl - 1) * 128 : Gl * 128], h0_d)

            hist_r = hist.rearrange("p (j c) -> p j c", c=128)

            def group(goff):
                # goff: token offset of this group (python int or ScalarValue)
                oh_t = sp.tile([V, Gl * BC], dt_w, tag="ohst")
                nc.sync.dma_start(oh_t[:], oh_d[:, ds(goff * BC, Gl * BC)])
                for j in range(Gl):
                    prev = (j - 1) % Gl
                    zp_a = pz.tile([128, 4 * BC], F32, tag="zpa")
                    zp_b = pz.tile([128, 4 * BC], F32, tag="zpb")
                    for m in range(MH):
                        zp = zp_a if m < 4 else zp_b
                        o = (m % 4) * BC
                        for k in range(KH):
                            nc.tensor.matmul(
                                zp[:, o : o + BC],
                                wh_s[:, k * H + m * 128 : k * H + (m + 1) * 128],
                                hist[:, prev * 128 + k * BC : prev * 128 + (k + 1) * BC],
                                start=(k == 0),
                                stop=False,
                            )
                        nc.tensor.matmul(
                            zp[:, o : o + BC],
                            mt_s[:, m * 128 : (m + 1) * 128],
                            oh_t[:, j * BC : (j + 1) * BC],
                            start=False,
                            stop=True,
                        )
                    nc.scalar.activation(
                        hist[:, j * 128 : j * 128 + 4 * BC],
                        zp_a[:],
                        mybir.ActivationFunctionType.Tanh,
                    )
                    nc.scalar.activation(
                        hist[:, j * 128 + 4 * BC : (j + 1) * 128],
                        zp_b[:],
                        mybir.ActivationFunctionType.Tanh,
                    )
                # W_o over the whole group's history ring
                lp = pl.tile([V, Gl * BC], F32, tag="lp")
                for k in range(KH):
                    nc.tensor.matmul(
                        lp[:],
                        wo_s[:, k * V : (k + 1) * V],
                        hist_r[:, :, k * BC : (k + 1) * BC],
                        start=(k == 0),
                        stop=(k == KH - 1),
                    )
                lg_t = sp.tile([V, Gl * BC], F32, tag="lgst")
                nc.vector.tensor_copy(lg_t[:], lp[:])
                nc.sync.dma_start(lg_d[:, ds(goff * BC, Gl * BC)], lg_t[:])

            if unroll == "full":
                for g in range(NG):
                    group(g * Gl)
            else:
                with tc.For_i(0, NG) as g:
                    group(g * Gl)

            nc.sync.dma_start(hf_d, hist[:, (Gl - 1) * 128 : Gl * 128])

    nc.compile()
    _cache[key] = nc
    return nc


def _prep_inputs(x, hidden, embedding, W_e, W_h, b_h, W_o, b_o, L_=L, dt_w=DT_W, dt_h=DT_H):
    npw, nph = _np_dt(dt_w), _np_dt(dt_h)
    x = np.asarray(x).astype(np.int64)
    hidden = np.asarray(hidden, dtype=np.float32)
    Mtable = (
        np.asarray(embedding, np.float32) @ np.asarray(W_e, np.float32)
        + np.asarray(b_h, np.float32)[None, :]
    )  # [V, H]
    W_h = np.asarray(W_h, np.float32)
    W_o = np.asarray(W_o, np.float32)
    # wh_blk[p, k*H + m*128 + c] = W_h[k*128+p, m*128+c]
    wh_blk = np.ascontiguousarray(
        W_h.reshape(KH, 128, MH, 128).transpose(1, 0, 2, 3).reshape(128, KH * H)
    ).astype(npw)
    # wo_blk[p, k*V + v] = W_o[k*128+p, v]
    wo_blk = np.ascontiguousarray(
        W_o.reshape(KH, 128, V).transpose(1, 0, 2).reshape(128, KH * V)
    ).astype(npw)
    mt = Mtable.astype(npw)
    in_maps = []
    for c in range(NCORES):
        xc = x[c * BC : (c + 1) * BC, :L_]  # [BC, L]
        # oh[v, t*BC + b] = (xc[b, t] == v)
        oh = (
            (xc.T[None, :, :] == np.arange(V, dtype=np.int64)[:, None, None])
            .reshape(V, L_ * BC)
            .astype(npw)
        )
        hc = hidden[c * BC : (c + 1) * BC]  # [BC, H]
        # h0[p, k*BC + b] = hc[b, k*128+p]
        h0 = np.ascontiguousarray(
            hc.reshape(BC, KH, 128).transpose(2, 1, 0).reshape(128, KH * BC)
        ).astype(nph)
        in_maps.append({"wh": wh_blk, "mt": mt, "wo": wo_blk, "oh": oh, "h0": h0})
    return in_maps


def _assemble(results, b_o, L_=L):
    b_o = np.asarray(b_o, np.float32)
    logits = np.empty((B, L_, V), np.float32)
    final_hidden = np.empty((B, H), np.float32)
    for c in range(NCORES):
        lg = np.asarray(results[c]["logits"], np.float32)  # [V, L*BC]
        logits[c * BC : (c + 1) * BC] = lg.reshape(V, L_, BC).transpose(2, 1, 0)
        hf = np.asarray(results[c]["hfinal"]).astype(np.float32)  # [128, KH*BC]
        final_hidden[c * BC : (c + 1) * BC] = (
            hf.reshape(128, KH, BC).transpose(2, 1, 0).reshape(BC, H)
        )
    logits += b_o[None, None, :]
    return logits, final_hidden


def run(inputs, L_=L, dt_w=DT_W, dt_h=DT_H, unroll=UNROLL, trace=False, **kw):
    nc = _build(L_, dt_w, dt_h, unroll)
    in_maps = _prep_inputs(L_=L_, dt_w=dt_w, dt_h=dt_h, **inputs)
    res = run_bass_kernel_spmd(
        nc, in_maps, core_ids=list(range(NCORES)), trace=trace, **kw
    )
    out = _assemble(res.results, inputs["b_o"], L_=L_)
    return out, res


def kernel(**inputs):
    out, _ = run(inputs)
    return out


# revision 14
# speedup vs baseline: 1.3992x; 1.3992x over previous
"""CharRNN Trainium2 kernel (data-parallel over batch, 8 NeuronCores).

Math (per core, batch slice of BC=16 rows):
  Mtable = embedding @ W_e + b_h                      [V, H]   (host)
  z_t^T  = W_h^T h_t^T + Mtable^T onehot(x_t)^T       [H, BC]
  h_{t+1}^T = tanh(z_t^T)
  logits_t^T = W_o^T h_{t+1}^T                        [V, BC]

Everything on device runs in the transposed domain so the hidden state
lives as [H(partitions), BC(free)] and tanh/matmuls use full partitions.
The recurrence is PE weight-load bound: per step 8x8 W_h blocks + 8
Mtable blocks are loaded as stationary operands with the BC-column state
as the moving operand.  W_o is applied every G steps over the SBUF-held
history ring with N=G*BC moving columns (weight reuse).
"""

import os
import sys

import numpy as np

_REPO = "/opt/trn_rl_repo"
if os.path.isdir(_REPO) and _REPO not in sys.path:
    sys.path.insert(0, _REPO)

import ml_dtypes  # noqa: E402

import concourse.bass as bass  # noqa: E402
import concourse.mybir as mybir  # noqa: E402
import concourse.tile as tile  # noqa: E402
from concourse import bacc  # noqa: E402
from concourse.bass import ds  # noqa: E402
from concourse.bass_utils import run_bass_kernel_spmd  # noqa: E402

B, L, V, E, H = 128, 1024, 32, 256, 1024
NCORES = 8
BC = B // NCORES  # 16 batch rows per core
KH = H // 128  # 8 contraction chunks
MH = H // 128  # 8 output chunks
G = 32  # steps per W_o flush; G*BC = 512 = max moving free dim
F32 = mybir.dt.float32

# dtype knobs: weights/state precision on device
DT_W = mybir.dt.bfloat16  # W_h / Mtable / W_o / onehot
DT_H = mybir.dt.bfloat16  # hidden-state history ring
UNROLL = "loop"  # "loop" (For_i over groups) | "full" (static unroll)
SPLIT_C = 1  # column-tiling ways for the recurrence stationary loads (1|2|4)

_cache = {}


def _np_dt(dt):
    return {
        F32: np.float32,
        mybir.dt.bfloat16: ml_dtypes.bfloat16,
        mybir.dt.float8e4: ml_dtypes.float8_e4m3,
    }[dt]


def _build(L_=L, dt_w=DT_W, dt_h=DT_H, unroll=UNROLL, repeat=1, split_c=SPLIT_C, dt_wh=None):
    dt_wh = dt_wh or dt_w
    key = (L_, dt_w, dt_h, unroll, repeat, split_c, dt_wh)
    if key in _cache:
        return _cache[key]
    Gl = min(G, L_)
    NG = L_ // Gl
    assert NG * Gl == L_

    nc = bacc.Bacc(
        "TRN2", target_bir_lowering=False, debug=False, num_devices=NCORES
    )
    wh_d = nc.dram_tensor("wh", [128, KH * H], dt_wh, kind="ExternalInput").ap()
    mt_d = nc.dram_tensor("mt", [V, H], dt_w, kind="ExternalInput").ap()
    wo_d = nc.dram_tensor("wo", [128, KH * V], dt_w, kind="ExternalInput").ap()
    oh_d = nc.dram_tensor("oh", [V, L_ * BC], dt_w, kind="ExternalInput").ap()
    h0_d = nc.dram_tensor("h0", [128, KH * BC], dt_h, kind="ExternalInput").ap()
    lg_d = nc.dram_tensor("logits", [V, L_ * BC], F32, kind="ExternalOutput").ap()
    hf_d = nc.dram_tensor("hfinal", [128, KH * BC], dt_h, kind="ExternalOutput").ap()

    with tile.TileContext(nc) as tc:
        with (
            tc.tile_pool(name="const", bufs=1) as cp,
            tc.tile_pool(name="stage", bufs=2) as sp,
            tc.tile_pool(name="psz", bufs=2, space="PSUM") as pz,
            tc.tile_pool(name="psl", bufs=2, space="PSUM") as pl,
        ):
            wh_s = cp.tile([128, KH * H], dt_wh, tag="wh")
            mt_s = cp.tile([V, H], dt_w, tag="mt")
            wo_s = cp.tile([128, KH * V], dt_w, tag="wo")
            hist = cp.tile([128, Gl * 128], dt_h, tag="hist")
            nc.sync.dma_start(wh_s[:], wh_d)
            nc.sync.dma_start(mt_s[:], mt_d)
            nc.sync.dma_start(wo_s[:], wo_d)
            nc.sync.dma_start(hist[:, (Gl - 1) * 128 : Gl * 128], h0_d)

            hist_r = hist.rearrange("p (j c) -> p j c", c=128)

            def group(goff):
                # goff: token offset of this group (python int or ScalarValue)
                oh_t = sp.tile([V, Gl * BC], dt_w, tag="ohst")
                nc.sync.dma_start(oh_t[:], oh_d[:, ds(goff * BC, Gl * BC)])
                for j in range(Gl):
                    prev = (j - 1) % Gl
                    zp_a = pz.tile([128, 4 * BC], F32, tag="zpa")
                    zp_b = pz.tile([128, 4 * BC], F32, tag="zpb")
                    cw = 128 // split_c  # stationary columns per strip
                    oh_j = oh_t[:, j * BC : (j + 1) * BC]
                    for m in range(MH):
                        zp = zp_a if m < 4 else zp_b
                        o = (m % 4) * BC
                        # strip-outer: each strip's accumulation group is
                        # sequential per bank (start=True zeroes a whole 2KB
                        # region); LDW concurrency across col-groups comes
                        # from the PE reorder window pulling loads ahead.
                        for c in range(split_c):
                            tp = (0, c * cw) if split_c > 1 else None
                            zps = zp[c * cw : (c + 1) * cw, o : o + BC]
                            for k in range(KH):
                                nc.tensor.matmul(
                                    zps,
                                    wh_s[
                                        :,
                                        k * H + m * 128 + c * cw : k * H
                                        + m * 128
                                        + (c + 1) * cw,
                                    ],
                                    hist[
                                        :,
                                        prev * 128 + k * BC : prev * 128 + (k + 1) * BC,
                                    ],
                                    start=(k == 0),
                                    stop=False,
                                    tile_position=tp,
                                )
                            nc.tensor.matmul(
                                zps,
                                mt_s[:, m * 128 + c * cw : m * 128 + (c + 1) * cw],
                                oh_j,
                                start=False,
                                stop=True,
                                tile_position=tp,
                            )
                    nc.scalar.activation(
                        hist[:, j * 128 : j * 128 + 4 * BC],
                        zp_a[:],
                        mybir.ActivationFunctionType.Tanh,
                    )
                    nc.scalar.activation(
                        hist[:, j * 128 + 4 * BC : (j + 1) * 128],
                        zp_b[:],
                        mybir.ActivationFunctionType.Tanh,
                    )
                # W_o over the whole group's history ring
                lp = pl.tile([V, Gl * BC], F32, tag="lp")
                for k in range(KH):
                    nc.tensor.matmul(
                        lp[:],
                        wo_s[:, k * V : (k + 1) * V],
                        hist_r[:, :, k * BC : (k + 1) * BC],
                        start=(k == 0),
                        stop=(k == KH - 1),
                    )
                lg_t = sp.tile([V, Gl * BC], F32, tag="lgst")
                nc.vector.tensor_copy(lg_t[:], lp[:])
                nc.sync.dma_start(lg_d[:, ds(goff * BC, Gl * BC)], lg_t[:])

            for _rep in range(repeat):
                if unroll == "full":
                    for g in range(NG):
                        group(g * Gl)
                else:
                    with tc.For_i(0, NG) as g:
                        group(g * Gl)

            nc.sync.dma_start(hf_d, hist[:, (Gl - 1) * 128 : Gl * 128])

    nc.compile()
    _cache[key] = nc
    return nc


def _prep_inputs(x, hidden, embedding, W_e, W_h, b_h, W_o, b_o, L_=L, dt_w=DT_W, dt_h=DT_H, dt_wh=None):
    npw, nph = _np_dt(dt_w), _np_dt(dt_h)
    npwh = _np_dt(dt_wh or dt_w)
    x = np.asarray(x).astype(np.int64)
    hidden = np.asarray(hidden, dtype=np.float32)
    Mtable = (
        np.asarray(embedding, np.float32) @ np.asarray(W_e, np.float32)
        + np.asarray(b_h, np.float32)[None, :]
    )  # [V, H]
    W_h = np.asarray(W_h, np.float32)
    W_o = np.asarray(W_o, np.float32)
    # wh_blk[p, k*H + m*128 + c] = W_h[k*128+p, m*128+c]
    wh_blk = np.ascontiguousarray(
        W_h.reshape(KH, 128, MH, 128).transpose(1, 0, 2, 3).reshape(128, KH * H)
    ).astype(npwh)
    # wo_blk[p, k*V + v] = W_o[k*128+p, v]
    wo_blk = np.ascontiguousarray(
        W_o.reshape(KH, 128, V).transpose(1, 0, 2).reshape(128, KH * V)
    ).astype(npw)
    mt = Mtable.astype(npw)
    in_maps = []
    for c in range(NCORES):
        xc = x[c * BC : (c + 1) * BC, :L_]  # [BC, L]
        # oh[v, t*BC + b] = (xc[b, t] == v)
        oh = (
            (xc.T[None, :, :] == np.arange(V, dtype=np.int64)[:, None, None])
            .reshape(V, L_ * BC)
            .astype(npw)
        )
        hc = hidden[c * BC : (c + 1) * BC]  # [BC, H]
        # h0[p, k*BC + b] = hc[b, k*128+p]
        h0 = np.ascontiguousarray(
            hc.reshape(BC, KH, 128).transpose(2, 1, 0).reshape(128, KH * BC)
        ).astype(nph)
        in_maps.append({"wh": wh_blk, "mt": mt, "wo": wo_blk, "oh": oh, "h0": h0})
    return in_maps


def _assemble(results, b_o, L_=L):
    b_o = np.asarray(b_o, np.float32)
    logits = np.empty((B, L_, V), np.float32)
    final_hidden = np.empty((B, H), np.float32)
    for c in range(NCORES):
        lg = np.asarray(results[c]["logits"], np.float32)  # [V, L*BC]
        logits[c * BC : (c + 1) * BC] = lg.reshape(V, L_, BC).transpose(2, 1, 0)
        hf = np.asarray(results[c]["hfinal"]).astype(np.float32)  # [128, KH*BC]
        final_hidden[c * BC : (c + 1) * BC] = (
            hf.reshape(128, KH, BC).transpose(2, 1, 0).reshape(BC, H)
        )
    logits += b_o[None, None, :]
    return logits, final_hidden


_runners = {}


def _get_runner(nc):
    """Build (once) a jitted shard_map executor for this program.

    Mirrors bass2jax.run_bass_via_pjrt's multi-core path but holds the
    jitted function so later calls pay no retrace/NEFF reload.
    """
    key = id(nc)
    if key in _runners:
        return _runners[key]
    import jax
    from jax.sharding import Mesh, PartitionSpec
    from jax.experimental.shard_map import shard_map
    from concourse import bass2jax
    from concourse.bass2jax import _bass_exec_p, install_neuronx_cc_hook

    install_neuronx_cc_hook()
    partition_name = nc.partition_id_tensor.name if nc.partition_id_tensor else None
    in_names, out_names, out_avals, out_shapes = [], [], [], []
    for alloc in nc.m.functions[0].allocations:
        if not isinstance(alloc, mybir.MemoryLocationSet):
            continue
        name = alloc.memorylocations[0].name
        if alloc.kind == "ExternalInput":
            if name != partition_name:
                in_names.append(name)
        elif alloc.kind == "ExternalOutput":
            shape = tuple(alloc.tensor_shape)
            dtype = mybir.dt.np(alloc.dtype)
            out_names.append(name)
            out_avals.append(jax.core.ShapedArray(shape, dtype))
            out_shapes.append((shape, dtype))
    all_in_names = list(in_names) + list(out_names)
    if partition_name is not None:
        all_in_names.append(partition_name)

    def _body(*args):
        operands = list(args)
        if partition_name is not None:
            operands.append(bass2jax.partition_id_tensor())
        outs = _bass_exec_p.bind(
            *operands,
            out_avals=tuple(out_avals),
            in_names=tuple(all_in_names),
            out_names=tuple(out_names),
            lowering_input_output_aliases=(),
            sim_require_finite=True,
            sim_require_nnan=True,
            nc=nc,
        )
        return tuple(outs)

    devices = jax.devices()[:NCORES]
    mesh = Mesh(np.asarray(devices), ("core",))
    n_io = len(in_names) + len(out_names)
    sharded = jax.jit(
        shard_map(
            _body,
            mesh=mesh,
            in_specs=(PartitionSpec("core"),) * n_io,
            out_specs=(PartitionSpec("core"),) * len(out_names),
            check_rep=False,
        ),
        keep_unused=True,
    )

    def runner(in_maps):
        import jax as _jax

        concat_in = [
            np.concatenate([np.asarray(m[n]) for m in in_maps], axis=0)
            for n in in_names
        ]
        concat_zero = [
            np.zeros((NCORES * s[0], *s[1:]), d) for (s, d) in out_shapes
        ]
        outs = sharded(*concat_in, *concat_zero)
        outs = [np.asarray(o) for o in _jax.block_until_ready(outs)]
        return [
            {
                n: outs[i].reshape(NCORES, *out_shapes[i][0])[c]
                for i, n in enumerate(out_names)
            }
            for c in range(NCORES)
        ]

    _runners[key] = runner
    return runner


class _Res:
    def __init__(self, results):
        self.results = results
        self.exec_time_ns = None


def run(inputs, L_=L, dt_w=DT_W, dt_h=DT_H, unroll=UNROLL, trace=False, repeat=1, split_c=SPLIT_C, dt_wh=None, use_cached_runner=True, **kw):
    nc = _build(L_, dt_w, dt_h, unroll, repeat, split_c, dt_wh)
    in_maps = _prep_inputs(L_=L_, dt_w=dt_w, dt_h=dt_h, dt_wh=dt_wh, **inputs)
    if use_cached_runner and not trace:
        results = _get_runner(nc)(in_maps)
        res = _Res(results)
    else:
        res = run_bass_kernel_spmd(
            nc, in_maps, core_ids=list(range(NCORES)), trace=trace, **kw
        )
        results = res.results
    out = _assemble(results, inputs["b_o"], L_=L_)
    return out, res


def kernel(**inputs):
    out, _ = run(inputs)
    return out


# revision 15
# speedup vs baseline: 186.4075x; 133.2285x over previous
"""CharRNN Trainium2 kernel (data-parallel over batch, 8 NeuronCores).

Math (per core, batch slice of BC=16 rows):
  Mtable = embedding @ W_e + b_h                      [V, H]   (host)
  z_t^T  = W_h^T h_t^T + Mtable^T onehot(x_t)^T       [H, BC]
  h_{t+1}^T = tanh(z_t^T)
  logits_t^T = W_o^T h_{t+1}^T                        [V, BC]

Everything on device runs in the transposed domain so the hidden state
lives as [H(partitions), BC(free)] and tanh/matmuls use full partitions.
The recurrence is PE weight-load bound: per step 8x8 W_h blocks + 8
Mtable blocks are loaded as stationary operands with the BC-column state
as the moving operand.  W_o is applied every G steps over the SBUF-held
history ring with N=G*BC moving columns (weight reuse).
"""

import os
import sys

import numpy as np

_REPO = "/opt/trn_rl_repo"
if os.path.isdir(_REPO) and _REPO not in sys.path:
    sys.path.insert(0, _REPO)

import ml_dtypes  # noqa: E402

import concourse.bass as bass  # noqa: E402
import concourse.mybir as mybir  # noqa: E402
import concourse.tile as tile  # noqa: E402
from concourse import bacc  # noqa: E402
from concourse.bass import ds  # noqa: E402
from concourse.bass_utils import run_bass_kernel_spmd  # noqa: E402

B, L, V, E, H = 128, 1024, 32, 256, 1024
NCORES = 8
BC = B // NCORES  # 16 batch rows per core
KH = H // 128  # 8 contraction chunks
MH = H // 128  # 8 output chunks
G = 32  # steps per W_o flush; G*BC = 512 = max moving free dim
F32 = mybir.dt.float32

# dtype knobs: weights/state precision on device
DT_W = mybir.dt.bfloat16  # W_h / Mtable / W_o / onehot
DT_H = mybir.dt.bfloat16  # hidden-state history ring
UNROLL = "loop"  # "loop" (For_i over groups) | "full" (static unroll)
SPLIT_C = 1  # column-tiling ways for the recurrence stationary loads (1|2|4)

_cache = {}


def _np_dt(dt):
    return {
        F32: np.float32,
        mybir.dt.bfloat16: ml_dtypes.bfloat16,
        mybir.dt.float8e4: ml_dtypes.float8_e4m3,
    }[dt]


def _build(L_=L, dt_w=DT_W, dt_h=DT_H, unroll=UNROLL, repeat=1, split_c=SPLIT_C, dt_wh=None):
    dt_wh = dt_wh or dt_w
    key = (L_, dt_w, dt_h, unroll, repeat, split_c, dt_wh)
    if key in _cache:
        return _cache[key]
    Gl = min(G, L_)
    NG = L_ // Gl
    assert NG * Gl == L_

    nc = bacc.Bacc(
        "TRN2", target_bir_lowering=False, debug=False, num_devices=NCORES
    )
    wh_d = nc.dram_tensor("wh", [128, KH * H], dt_wh, kind="ExternalInput").ap()
    mt_d = nc.dram_tensor("mt", [V, H], dt_w, kind="ExternalInput").ap()
    wo_d = nc.dram_tensor("wo", [128, KH * V], dt_w, kind="ExternalInput").ap()
    oh_d = nc.dram_tensor("oh", [V, L_ * BC], dt_w, kind="ExternalInput").ap()
    h0_d = nc.dram_tensor("h0", [128, KH * BC], dt_h, kind="ExternalInput").ap()
    lg_d = nc.dram_tensor("logits", [V, L_ * BC], F32, kind="ExternalOutput").ap()
    hf_d = nc.dram_tensor("hfinal", [128, KH * BC], dt_h, kind="ExternalOutput").ap()

    with tile.TileContext(nc) as tc:
        with (
            tc.tile_pool(name="const", bufs=1) as cp,
            tc.tile_pool(name="stage", bufs=2) as sp,
            tc.tile_pool(name="psz", bufs=2, space="PSUM") as pz,
            tc.tile_pool(name="psl", bufs=2, space="PSUM") as pl,
        ):
            wh_s = cp.tile([128, KH * H], dt_wh, tag="wh")
            mt_s = cp.tile([V, H], dt_w, tag="mt")
            wo_s = cp.tile([128, KH * V], dt_w, tag="wo")
            hist = cp.tile([128, Gl * 128], dt_h, tag="hist")
            nc.sync.dma_start(wh_s[:], wh_d)
            nc.sync.dma_start(mt_s[:], mt_d)
            nc.sync.dma_start(wo_s[:], wo_d)
            nc.sync.dma_start(hist[:, (Gl - 1) * 128 : Gl * 128], h0_d)

            hist_r = hist.rearrange("p (j c) -> p j c", c=128)

            def group(goff):
                # goff: token offset of this group (python int or ScalarValue)
                oh_t = sp.tile([V, Gl * BC], dt_w, tag="ohst")
                nc.sync.dma_start(oh_t[:], oh_d[:, ds(goff * BC, Gl * BC)])
                for j in range(Gl):
                    prev = (j - 1) % Gl
                    zp_a = pz.tile([128, 4 * BC], F32, tag="zpa")
                    zp_b = pz.tile([128, 4 * BC], F32, tag="zpb")
                    cw = 128 // split_c  # stationary columns per strip
                    oh_j = oh_t[:, j * BC : (j + 1) * BC]
                    for m in range(MH):
                        zp = zp_a if m < 4 else zp_b
                        o = (m % 4) * BC
                        # strip-outer: each strip's accumulation group is
                        # sequential per bank (start=True zeroes a whole 2KB
                        # region); LDW concurrency across col-groups comes
                        # from the PE reorder window pulling loads ahead.
                        for c in range(split_c):
                            tp = (0, c * cw) if split_c > 1 else None
                            zps = zp[c * cw : (c + 1) * cw, o : o + BC]
                            for k in range(KH):
                                nc.tensor.matmul(
                                    zps,
                                    wh_s[
                                        :,
                                        k * H + m * 128 + c * cw : k * H
                                        + m * 128
                                        + (c + 1) * cw,
                                    ],
                                    hist[
                                        :,
                                        prev * 128 + k * BC : prev * 128 + (k + 1) * BC,
                                    ],
                                    start=(k == 0),
                                    stop=False,
                                    tile_position=tp,
                                )
                            nc.tensor.matmul(
                                zps,
                                mt_s[:, m * 128 + c * cw : m * 128 + (c + 1) * cw],
                                oh_j,
                                start=False,
                                stop=True,
                                tile_position=tp,
                            )
                    nc.scalar.activation(
                        hist[:, j * 128 : j * 128 + 4 * BC],
                        zp_a[:],
                        mybir.ActivationFunctionType.Tanh,
                    )
                    nc.scalar.activation(
                        hist[:, j * 128 + 4 * BC : (j + 1) * 128],
                        zp_b[:],
                        mybir.ActivationFunctionType.Tanh,
                    )
                # W_o over the whole group's history ring
                lp = pl.tile([V, Gl * BC], F32, tag="lp")
                for k in range(KH):
                    nc.tensor.matmul(
                        lp[:],
                        wo_s[:, k * V : (k + 1) * V],
                        hist_r[:, :, k * BC : (k + 1) * BC],
                        start=(k == 0),
                        stop=(k == KH - 1),
                    )
                lg_t = sp.tile([V, Gl * BC], F32, tag="lgst")
                nc.vector.tensor_copy(lg_t[:], lp[:])
                nc.sync.dma_start(lg_d[:, ds(goff * BC, Gl * BC)], lg_t[:])

            for _rep in range(repeat):
                if unroll == "full":
                    for g in range(NG):
                        group(g * Gl)
                else:
                    with tc.For_i(0, NG) as g:
                        group(g * Gl)

            nc.sync.dma_start(hf_d, hist[:, (Gl - 1) * 128 : Gl * 128])

    nc.compile()
    _cache[key] = nc
    return nc


def _prep_inputs(x, hidden, embedding, W_e, W_h, b_h, W_o, b_o, L_=L, dt_w=DT_W, dt_h=DT_H, dt_wh=None):
    npw, nph = _np_dt(dt_w), _np_dt(dt_h)
    npwh = _np_dt(dt_wh or dt_w)
    x = np.asarray(x).astype(np.int64)
    hidden = np.asarray(hidden, dtype=np.float32)
    Mtable = (
        np.asarray(embedding, np.float32) @ np.asarray(W_e, np.float32)
        + np.asarray(b_h, np.float32)[None, :]
    )  # [V, H]
    W_h = np.asarray(W_h, np.float32)
    W_o = np.asarray(W_o, np.float32)
    # wh_blk[p, k*H + m*128 + c] = W_h[k*128+p, m*128+c]
    wh_blk = np.ascontiguousarray(
        W_h.reshape(KH, 128, MH, 128).transpose(1, 0, 2, 3).reshape(128, KH * H)
    ).astype(npwh)
    # wo_blk[p, k*V + v] = W_o[k*128+p, v]
    wo_blk = np.ascontiguousarray(
        W_o.reshape(KH, 128, V).transpose(1, 0, 2).reshape(128, KH * V)
    ).astype(npw)
    mt = Mtable.astype(npw)
    in_maps = []
    for c in range(NCORES):
        xc = x[c * BC : (c + 1) * BC, :L_]  # [BC, L]
        # oh[v, t*BC + b] = (xc[b, t] == v), built by scatter
        oh = np.zeros((V, L_ * BC), npw)
        oh[xc.T.ravel(), np.arange(L_ * BC)] = 1
        hc = hidden[c * BC : (c + 1) * BC]  # [BC, H]
        # h0[p, k*BC + b] = hc[b, k*128+p]
        h0 = np.ascontiguousarray(
            hc.reshape(BC, KH, 128).transpose(2, 1, 0).reshape(128, KH * BC)
        ).astype(nph)
        in_maps.append({"wh": wh_blk, "mt": mt, "wo": wo_blk, "oh": oh, "h0": h0})
    return in_maps


def _assemble(results, b_o, L_=L):
    b_o = np.asarray(b_o, np.float32)
    logits = np.empty((B, L_, V), np.float32)
    final_hidden = np.empty((B, H), np.float32)
    for c in range(NCORES):
        lg = np.asarray(results[c]["logits"], np.float32)  # [V, L*BC]
        logits[c * BC : (c + 1) * BC] = lg.reshape(V, L_, BC).transpose(2, 1, 0)
        hf = np.asarray(results[c]["hfinal"]).astype(np.float32)  # [128, KH*BC]
        final_hidden[c * BC : (c + 1) * BC] = (
            hf.reshape(128, KH, BC).transpose(2, 1, 0).reshape(BC, H)
        )
    logits += b_o[None, None, :]
    return logits, final_hidden


_runners = {}


def _get_runner(nc):
    """Build (once) a jitted shard_map executor for this program.

    Mirrors bass2jax.run_bass_via_pjrt's multi-core path but holds the
    jitted function so later calls pay no retrace/NEFF reload.
    """
    key = id(nc)
    if key in _runners:
        return _runners[key]
    import jax
    from jax.sharding import Mesh, PartitionSpec
    from jax.experimental.shard_map import shard_map
    from concourse import bass2jax
    from concourse.bass2jax import _bass_exec_p, install_neuronx_cc_hook

    install_neuronx_cc_hook()
    partition_name = nc.partition_id_tensor.name if nc.partition_id_tensor else None
    in_names, out_names, out_avals, out_shapes = [], [], [], []
    for alloc in nc.m.functions[0].allocations:
        if not isinstance(alloc, mybir.MemoryLocationSet):
            continue
        name = alloc.memorylocations[0].name
        if alloc.kind == "ExternalInput":
            if name != partition_name:
                in_names.append(name)
        elif alloc.kind == "ExternalOutput":
            shape = tuple(alloc.tensor_shape)
            dtype = mybir.dt.np(alloc.dtype)
            out_names.append(name)
            out_avals.append(jax.core.ShapedArray(shape, dtype))
            out_shapes.append((shape, dtype))
    all_in_names = list(in_names) + list(out_names)
    if partition_name is not None:
        all_in_names.append(partition_name)

    def _body(*args):
        operands = list(args)
        if partition_name is not None:
            operands.append(bass2jax.partition_id_tensor())
        outs = _bass_exec_p.bind(
            *operands,
            out_avals=tuple(out_avals),
            in_names=tuple(all_in_names),
            out_names=tuple(out_names),
            lowering_input_output_aliases=(),
            sim_require_finite=True,
            sim_require_nnan=True,
            nc=nc,
        )
        return tuple(outs)

    devices = jax.devices()[:NCORES]
    mesh = Mesh(np.asarray(devices), ("core",))
    n_io = len(in_names) + len(out_names)
    sharded = jax.jit(
        shard_map(
            _body,
            mesh=mesh,
            in_specs=(PartitionSpec("core"),) * n_io,
            out_specs=(PartitionSpec("core"),) * len(out_names),
            check_rep=False,
        ),
        keep_unused=True,
    )

    def runner(in_maps):
        import jax as _jax

        concat_in = [
            np.concatenate([np.asarray(m[n]) for m in in_maps], axis=0)
            for n in in_names
        ]
        concat_zero = [
            np.zeros((NCORES * s[0], *s[1:]), d) for (s, d) in out_shapes
        ]
        outs = sharded(*concat_in, *concat_zero)
        outs = [np.asarray(o) for o in _jax.block_until_ready(outs)]
        return [
            {
                n: outs[i].reshape(NCORES, *out_shapes[i][0])[c]
                for i, n in enumerate(out_names)
            }
            for c in range(NCORES)
        ]

    _runners[key] = runner
    return runner


class _Res:
    def __init__(self, results):
        self.results = results
        self.exec_time_ns = None


def run(inputs, L_=L, dt_w=DT_W, dt_h=DT_H, unroll=UNROLL, trace=False, repeat=1, split_c=SPLIT_C, dt_wh=None, use_cached_runner=True, **kw):
    nc = _build(L_, dt_w, dt_h, unroll, repeat, split_c, dt_wh)
    in_maps = _prep_inputs(L_=L_, dt_w=dt_w, dt_h=dt_h, dt_wh=dt_wh, **inputs)
    if use_cached_runner and not trace:
        results = _get_runner(nc)(in_maps)
        res = _Res(results)
    else:
        res = run_bass_kernel_spmd(
            nc, in_maps, core_ids=list(range(NCORES)), trace=trace, **kw
        )
        results = res.results
    out = _assemble(results, inputs["b_o"], L_=L_)
    return out, res


def kernel(**inputs):
    out, _ = run(inputs)
    return out
